# revision 19
# baseline (speedup 1.0000x reference)
"""CrossAttention (RoPE, 16 heads, C=1024) Trainium2 Bass kernel, v3.

Sharding: DP over batch (4) x TP over heads (2 groups of 8) = 8 cores.

Numerics (error-budgeted against the 2e-2 gate):
- Q/K projections: fp8e4m3 DoubleRow with hi+lo input decomposition
  (2048 effective contraction rows), weights fp8*WS.
- RoPE via the frequency-symmetry identity: the k side needs NO rotation:
    S = sum_d kcos[d]*qpl0[d] + ksin[d]*qpl1[d]
  with kcos = k^*cos_k, ksin = k^*sin_k (plain muls straight from psum),
    qpl0 = q^*cos_q + rot(q^*sin_q),  qpl1 = q^*sin_q - rot(q^*cos_q).
  Only the 4 q-tiles take PE rotate matmuls.
- Scores: fp8 DR, 2 real planes (kcos|ksin stationary, qpl0|qpl1 moving).
  exp on ACT -> fp16 probs.
- V projection: fp8 DR with v hi+lo as extra contraction chunks, wv
  fp8*WS moving; psum evac * (1/WS) -> fp16 v65 (+ones col for rowsums).
- PV/out-proj: fp16 (fp8 probs would blow the error budget), late
  normalization, PE transposes.
"""

import sys

if "/opt/trn_rl_repo" not in sys.path:
    sys.path.insert(0, "/opt/trn_rl_repo")

import numpy as np
import ml_dtypes
from contextlib import ExitStack

import concourse.bass as bass
import concourse.tile as tile
from concourse import bacc, mybir

from concourse.dve_spec import Spec, Src0, C0, C1, C2, One, sq, lower
from concourse import dve_ops as _dve_ops
from concourse.dve_ops import DveOp


def _register_exp_ops():
    if "EXP_POLY_A" in _dve_ops._SUB_OPCODE_FOR_NAME:
        return (_dve_ops.CUSTOM_DVE_SPECS["EXP_POLY_A"],)


def _np32(x):
    return np.asarray(x, np.float32)


def _ref_exp_a(in0, in1, c0, c1, c2):
    w = _np32(_np32(in0) * np.float32(c0))
    t = _np32(w * np.float32(c1))
    t2 = _np32(t + np.float32(c2))
    w2 = _np32(w * w)
    u = _np32(w2 * t2)
    v = _np32(u + w)
    return _np32(v + np.float32(1.0))


def _ref_exp_b(in0, in1, c0, c1, c2):
    x = _np32(in0)
    for _ in range(4):
        x = _np32(x * x)
    return x


_w = Src0 * C0
_EXP_A_BODY = (sq(_w) * (_w * C1 + C2) + _w) + One
_x2 = sq(Src0)
_EXP_B_BODY = sq(sq(sq(sq(Src0))))

EXP_POLY_A = DveOp(
    "EXP_POLY_A",
    Spec(body=_EXP_A_BODY, reference=_ref_exp_a),
    subdim=False,
    uops_sha={},
)
EXP_POLY_B = DveOp(
    "EXP_POLY_B",
    Spec(body=_EXP_B_BODY, reference=_ref_exp_b),
    subdim=False,
    uops_sha={},
)

for _op in (EXP_POLY_A, EXP_POLY_B):
    if _op.name not in _dve_ops._SUB_OPCODE_FOR_NAME:
        _dve_ops.OPS.append(_op)
        _dve_ops._SUB_OPCODE_FOR_NAME[_op.name] = (
            _dve_ops._CUSTOM_DVE_ROW_BASE + len(_dve_ops.OPS) - 1)
        _dve_ops.CUSTOM_DVE_SPECS[_op.name] = _op.spec
    for _ver in ("v3", "v4"):
        try:
            _op.compile(_ver)
        except ValueError as e:
            import re as _re
            _mm = _re.search(r'uops_sha\["' + _ver + r'"\]="([0-9a-f]+)"',
                             str(e))
            if not _mm:
                raise
            _op.uops_sha[_ver] = _mm.group(1)
            _op.compile(_ver)

F32 = mybir.dt.float32
F16 = mybir.dt.float16
F8 = mybir.dt.float8e4
DR = mybir.MatmulPerfMode.DoubleRow
EXP = mybir.ActivationFunctionType.Exp
E4NP = ml_dtypes.float8_e4m3
F16NP = np.float16

# problem constants
B, Nq, Nk, C = 4, 512, 2048, 1024
H, D = 16, 64
HL = 8            # heads per core
CH = HL * D       # 512 local channels
NPAIR = HL // 2   # 4 pairs of heads (128 rows each)
SB = Nk // 512    # 4 key blocks of 512
SC = Nk // 128    # 16 key chunks of 128
ROPE_BASE = 10000.0
SCALE = float(D) ** -0.5
WS = 32.0         # fp8 weight prescale (32 keeps qpl/kpl < e4m3 max)
ESC = SCALE / (WS * WS)   # folded into the exp activation scale
DEBUG = False
N_DVE_EXP = 10            # of 64 exp tiles, how many go to the DVE poly path


def _ld3(eng, dst_view, src_2d, width=512):
    """DMA a [N*128, width] DRAM region into a [128, N*width] tile view."""
    eng.dma_start(
        dst_view.rearrange("p (a s) -> p a s", s=width),
        src_2d.rearrange("(a p) s -> p a s", p=128))


def build_nc(iters: int = 1):
    nc = bacc.Bacc("TRN2", target_bir_lowering=False, debug=False)

    qhT = nc.dram_tensor("qhT", [C, Nq], F8, kind="ExternalInput")
    qlT = nc.dram_tensor("qlT", [C, Nq], F8, kind="ExternalInput")
    khT = nc.dram_tensor("khT", [C, Nk], F8, kind="ExternalInput")
    klT = nc.dram_tensor("klT", [C, Nk], F8, kind="ExternalInput")
    vhT = nc.dram_tensor("vhT", [C, Nk], F8, kind="ExternalInput")
    vlT = nc.dram_tensor("vlT", [C, Nk], F8, kind="ExternalInput")
    wqT = nc.dram_tensor("wqT", [C, CH], F8, kind="ExternalInput")
    wkT = nc.dram_tensor("wkT", [C, CH], F8, kind="ExternalInput")
    wvT = nc.dram_tensor("wvT", [C, CH], F8, kind="ExternalInput")
    wpT = nc.dram_tensor("wpT", [CH, C], F16, kind="ExternalInput")
    # cst = [rT | idn]; qtab = [cq | sq]; ktab = per-block [ck | sk]
    cst = nc.dram_tensor("cst", [128, 256], F16, kind="ExternalInput")
    qtab = nc.dram_tensor("qtab", [128, 2 * Nq], F16, kind="ExternalInput")
    ktab = nc.dram_tensor("ktab", [128, 2 * Nk], F16, kind="ExternalInput")
    outT = nc.dram_tensor("outT", [C, Nq], F32, kind="ExternalOutput")
    if DEBUG:
        d_qpl = nc.dram_tensor("d_qpl", [128, NPAIR * 1024], F8,
                               kind="ExternalOutput")
        d_kpl = nc.dram_tensor("d_kpl", [128, 2 * Nk], F8,
                               kind="ExternalOutput")
        d_v65 = nc.dram_tensor("d_v65", [128, SC * 520], F16,
                               kind="ExternalOutput")
        d_xn = nc.dram_tensor("d_xn", [128, HL * 4 * 64], F16,
                              kind="ExternalOutput")
        d_pt = nc.dram_tensor("d_pt", [128, 2048], F16,
                              kind="ExternalOutput")

    with tile.TileContext(nc) as tc, ExitStack() as top:
        const = top.enter_context(tc.tile_pool(name="const", bufs=1))
        cst_t = const.tile([128, 256], F16, tag="cst", name="cst")
        rt_t = cst_t[0:128, 0:128]
        id_t = cst_t[0:128, 128:256]
        cst_loaded = [False]

        for _ in range(iters):
            with ExitStack() as it:
                per = it.enter_context(tc.tile_pool(name="per", bufs=1))
                qpl = per.tile([128, NPAIR * 1024], F8, tag="qpl", name="qpl")
                kpl = [per.tile([128, 2 * Nk], F8, tag=f"kpl{m}",
                                name=f"kpl{m}") for m in range(NPAIR)]
                v65 = per.tile([128, SC * 520], F16, tag="v65", name="v65")
                inv_t = per.tile([128, HL * 4], F32, tag="inv", name="inv")
                xn = per.tile([128, HL * 4 * 64], F16, tag="xn", name="xn")
                # all 64 prob tiles live in one [128, 64K] fp16 tile;
                # (sbi, h, jj) -> columns [idx*1024, (idx+1)*1024)
                pts_t = per.tile([128, 64 * 1024], F16, tag="pts",
                                 name="pts")

                def pt_view(sbi, h, jj):
                    idx = sbi * 16 + h * 2 + jj
                    return pts_t[:, idx * 1024:(idx + 1) * 1024]

                stg = it.enter_context(ExitStack())
                ktm = stg.enter_context(tc.tile_pool(name="ktm", bufs=2))
                w8 = stg.enter_context(tc.tile_pool(name="w8", bufs=1))
                kst = stg.enter_context(tc.tile_pool(name="kst", bufs=1))
                ktb = stg.enter_context(tc.tile_pool(name="ktb", bufs=2))
                s0 = ExitStack()   # block-0-lifetime staging
                qst = s0.enter_context(tc.tile_pool(name="qst", bufs=1))
                qtb = s0.enter_context(tc.tile_pool(name="qtb", bufs=1))
                atm = s0.enter_context(tc.tile_pool(name="atm", bufs=2))
                wqp = s0.enter_context(tc.tile_pool(name="wqp", bufs=1))
                vs = ExitStack()   # v-projection-lifetime staging

                # ---- SP queue: k-side stream (gates pipeline start);
                #      ACT queue: q-side + v/p weights (ACT idle in lead) ----
                if not cst_loaded[0]:
                    nc.scalar.dma_start(cst_t[:], cst[:])
                    cst_loaded[0] = True
                wk_t = w8.tile([128, 8 * CH], F8, tag="w8", name="wk")
                _ld3(nc.sync, wk_t[:], wkT[:], CH)
                wq_t = wqp.tile([128, 8 * CH], F8, tag="wq", name="wq")
                _ld3(nc.scalar, wq_t[:], wqT[:], CH)
                qt_t = qst.tile([128, 2 * 4096], F8, tag="qt", name="qt")
                _ld3(nc.scalar, qt_t[:, 0:4096], qhT[:], 512)
                _ld3(nc.scalar, qt_t[:, 4096:8192], qlT[:], 512)
                qtab_t = qtb.tile([128, 2 * Nq], F16, tag="qtb", name="qtab")
                nc.scalar.dma_start(qtab_t[:], qtab[:])

                # ---- k/v block loads on the SP queue ----
                kt_ts, vt_ts, ktab_ts = [], [], []
                for sbi in range(SB):
                    sl = slice(sbi * 512, (sbi + 1) * 512)
                    kt_t = kst.tile([128, 8192], F8, tag="kt", name="kt")
                    _ld3(nc.sync, kt_t[:, 0:4096], khT[:, sl], 512)
                    _ld3(nc.sync, kt_t[:, 4096:8192], klT[:, sl], 512)
                    ktab_t = ktb.tile([128, 1024], F16, tag="ktb",
                                      name="ktab")
                    nc.sync.dma_start(
                        ktab_t[:], ktab[:, sbi * 1024:(sbi + 1) * 1024])
                    kt_ts.append(kt_t)
                    ktab_ts.append(ktab_t)

                def proj_hilo(pq, w_t, x_t, m, width):
                    wv_ = w_t[:].rearrange("p (a m) -> p a m", m=CH)
                    xv_ = x_t[:].rearrange("p (a n) -> p a n", n=width)
                    for s in range(8):
                        ws = s % 4
                        nc.tensor.matmul(
                            pq[:],
                            wv_[:, 2 * ws:2 * ws + 2, m * 128:(m + 1) * 128],
                            xv_[:, 2 * s:2 * s + 2, :],
                            start=(s == 0), stop=(s == 7), perf_mode=DR)

                def kproj_m(sbi, m, psum_pool):
                    pk = psum_pool.tile([128, 512], F32, tag="pk", name="pk")
                    proj_hilo(pk, wk_t, kt_ts[sbi], m, 512)
                    ktab_t = ktab_ts[sbi]
                    xk = ktm.tile([128, 512], F16, tag="xk", name="xk")
                    nc.vector.tensor_copy(xk[:], pk[:])
                    nc.gpsimd.tensor_mul(
                        kpl[m][:, sbi * 512:(sbi + 1) * 512],
                        xk[:], ktab_t[0:128, 0:512])
                    nc.gpsimd.tensor_mul(
                        kpl[m][:, Nk + sbi * 512:Nk + (sbi + 1) * 512],
                        xk[:], ktab_t[0:128, 512:1024])

                def qside_m(m, psum_pool, rot_pool):
                    pq = psum_pool.tile([128, 512], F32, tag="pk", name="pq")
                    proj_hilo(pq, wq_t, qt_t, m, Nq)
                    xsq = atm.tile([128, Nq], F16, tag="xsq", name="xsq")
                    nc.scalar.copy(xsq[:], pq[:])
                    m_c = atm.tile([128, Nq], F16, tag="mc", name="mc")
                    nc.gpsimd.tensor_mul(m_c[:], xsq[:], qtab_t[0:128, 0:Nq])
                    m_s = atm.tile([128, Nq], F16, tag="ms", name="ms")
                    nc.gpsimd.tensor_mul(m_s[:], xsq[:],
                                         qtab_t[0:128, Nq:2 * Nq])
                    prot0 = rot_pool.tile([128, Nq], F32, tag="prot",
                                          name="prot0")
                    nc.tensor.matmul(prot0[:], rt_t, m_s[:],
                                     start=True, stop=True)
                    nc.vector.tensor_add(
                        qpl[:, m * 1024:m * 1024 + 512], m_c[:], prot0[:])
                    prot1 = rot_pool.tile([128, Nq], F32, tag="prot",
                                          name="prot1")
                    nc.tensor.matmul(prot1[:], rt_t, m_c[:],
                                     start=True, stop=True)
                    nc.vector.tensor_sub(
                        qpl[:, m * 1024 + 512:(m + 1) * 1024],
                        m_s[:], prot1[:])

                def vproj_chunk(vb, scj, pvp):
                    vt_ = vt_ts[vb][:].rearrange("p (a n) -> p a n", n=512)
                    wv_ = wv_t[:].rearrange("p (a m) -> p a m", m=CH)
                    sc = vb * 4 + scj
                    pv = pvp.tile([128, CH], F32, tag="pv", name="pv")
                    for s in range(8):
                        ws = s % 4
                        nc.tensor.matmul(
                            pv[:],
                            vt_[:, 2 * s:2 * s + 2,
                                scj * 128:(scj + 1) * 128],
                            wv_[:, 2 * ws:2 * ws + 2, :],
                            start=(s == 0), stop=(s == 7), perf_mode=DR)
                    nc.vector.tensor_scalar_mul(
                        v65[:, sc * 520:(sc + 1) * 520
                            ].rearrange("p (n w) -> p n w",
                                        w=65)[:, :, 0:64],
                        pv[:].rearrange("p (n w) -> p n w", w=64),
                        1.0 / WS)

                def scores_h(sbi, h, scp):
                    m, r0 = h // 2, 64 * (h % 2)
                    stv = kpl[m][r0:r0 + 64, :].rearrange(
                        "p (two n) -> p two n", two=2)
                    mvv = qpl[r0:r0 + 64,
                              m * 1024:(m + 1) * 1024].rearrange(
                        "p (two n) -> p two n", two=2)
                    for jj in range(2):
                        psc = scp.tile([128, 1024], F32, tag="psc",
                                       name="psc")
                        for j2 in range(2):
                            sc = sbi * 4 + jj * 2 + j2
                            nc.tensor.matmul(
                                psc[:, j2 * 512:(j2 + 1) * 512],
                                stv[:, :, sc * 128:(sc + 1) * 128],
                                mvv, start=True, stop=True, perf_mode=DR)
                        pt = pt_view(sbi, h, jj)
                        idx = sbi * 16 + h * 2 + jj
                        if N_DVE_EXP > 0 and idx < 56 and idx % 5 == 1:
                            nc.vector._custom_dve(
                                EXP_POLY_A, out=psc[:], in0=psc[:],
                                s0=float(ESC / 16.0), s1=float(1.0 / 6.0),
                                imm2=0.5)
                            nc.vector._custom_dve(
                                EXP_POLY_B, out=pt, in0=psc[:])
                        else:
                            nc.scalar.activation(pt, psc[:], EXP, scale=ESC)
                        pts[(sbi, h, jj)] = pt

                pts = {}
                with ExitStack() as phb:
                    scp = phb.enter_context(
                        tc.tile_pool(name="scp", bufs=2, space="PSUM"))
                    pps_stack = ExitStack()
                    pps = pps_stack.enter_context(
                        tc.tile_pool(name="pps", bufs=2, space="PSUM"))
                    # block 0: per-pair weave; block-1 k-proj rides along
                    with ExitStack() as ph0:
                        rp0 = ph0.enter_context(
                            tc.tile_pool(name="rp0", bufs=2, space="PSUM"))
                        for m in range(NPAIR):
                            kproj_m(0, m, pps)
                            qside_m(m, pps, rp0)
                            scores_h(0, 2 * m, scp)
                            scores_h(0, 2 * m + 1, scp)
                    s0.close()
                    nc.vector.memset(
                        v65[:].rearrange("p (s h w) -> p s h w", h=HL,
                                         w=65)[:, :, :, 64:65], 1.0)
                    wvp = vs.enter_context(tc.tile_pool(name="wvp", bufs=1))
                    vst = vs.enter_context(tc.tile_pool(name="vst", bufs=2))
                    wv_t = wvp.tile([128, 8 * CH], F8, tag="wv", name="wv")
                    _ld3(nc.sync, wv_t[:], wvT[:], CH)
                    for _sbi in range(SB):
                        _sl = slice(_sbi * 512, (_sbi + 1) * 512)
                        vt_t = vst.tile([128, 8192], F8, tag="vt", name="vt")
                        _ld3(nc.sync, vt_t[:, 0:4096], vhT[:, _sl], 512)
                        _ld3(nc.sync, vt_t[:, 4096:8192], vlT[:, _sl], 512)
                        vt_ts.append(vt_t)
                    # blocks 1-2: scores woven with next-block k-proj and
                    # the v projections (thunk queue, ~1-2 per head)
                    with ExitStack() as phk:
                        pvp = phk.enter_context(
                            tc.tile_pool(name="pvp", bufs=2, space="PSUM"))
                        for m in range(NPAIR):
                            kproj_m(1, m, pps)
                        weaves = {
                            1: ([lambda m=m: kproj_m(2, m, pps)
                                 for m in range(NPAIR)]
                                + [lambda c=c: vproj_chunk(c // 4, c % 4,
                                                           pvp)
                                   for c in range(0, 8)]),
                            2: ([lambda c=c: vproj_chunk(c // 4, c % 4,
                                                         pvp)
                                 for c in range(8, 12)]
                                + [lambda m=m: kproj_m(3, m, pps)
                                   for m in range(NPAIR)]
                                + [lambda c=c: vproj_chunk(c // 4, c % 4,
                                                           pvp)
                                   for c in range(12, 16)]),
                        }
                        for sbi in (1, 2):
                            wq_ = weaves[sbi]
                            nper = (len(wq_) + HL - 1) // HL
                            for h in range(HL):
                                scores_h(sbi, h, scp)
                                for t in wq_[h * nper:(h + 1) * nper]:
                                    t()
                    vs.close()
                    stg.close()
                    pps_stack.close()
                    wpp = it.enter_context(tc.tile_pool(name="wpp", bufs=1))
                    wp_t = wpp.tile([128, NPAIR * 1024], F16, tag="wp",
                                    name="wp")
                    _ld3(nc.sync, wp_t[:], wpT[:], 1024)

                    # block-3 scores with PV/normalization/transposes woven
                    # in one head behind the exp wave
                    xtt = it.enter_context(
                        tc.tile_pool(name="xtt", bufs=1))
                    with ExitStack() as phx:
                        xtp = phx.enter_context(
                            tc.tile_pool(name="xtp", bufs=2, space="PSUM"))
                        xnT = [xtt.tile([128, Nq], F16, tag=f"xnT{p}",
                                        name=f"xnT{p}")
                               for p in range(NPAIR)]

                        def pv_h(h):
                            pxt = xtp.tile([128, 260], F32, tag="pxt",
                                           name="pxt",
                                           padded_shape=[128, 512])
                            for qc in range(4):
                                for sc in range(SC):
                                    sbi, jj, j2 = (sc // 4, (sc % 4) // 2,
                                                   sc % 2)
                                    nc.tensor.matmul(
                                        pxt[:, qc * 65:(qc + 1) * 65],
                                        pts[(sbi, h, jj)][
                                            :, j2 * 512 + qc * 128:
                                            j2 * 512 + (qc + 1) * 128],
                                        v65[:, sc * 520 + h * 65:
                                            sc * 520 + (h + 1) * 65],
                                        start=(sc == 0), stop=(sc == SC - 1))
                            nc.vector.reciprocal(
                                inv_t[:, h * 4:(h + 1) * 4].rearrange(
                                    "p (a b) -> p a b", b=1),
                                pxt[:].rearrange("p (q w) -> p q w",
                                                 w=65)[:, :, 64:65])
                            for qc in range(4):
                                nc.vector.tensor_scalar_mul(
                                    xn[:, (h * 4 + qc) * 64:
                                       (h * 4 + qc + 1) * 64],
                                    pxt[:, qc * 65:qc * 65 + 64],
                                    inv_t[:, h * 4 + qc:h * 4 + qc + 1])

                        def transp_p(p):
                            for qc in range(4):
                                ptf = xtp.tile([128, 256], F32, tag="pxt",
                                               name="ptr",
                                               padded_shape=[128, 512])
                                ptr = ptf[:].bitcast(F16)
                                for sub in range(2):
                                    hh = 2 * p + sub
                                    nc.tensor.transpose(
                                        ptr[sub * 64:(sub + 1) * 64, 0:128],
                                        xn[:, (hh * 4 + qc) * 64:
                                           (hh * 4 + qc + 1) * 64],
                                        id_t[:],
                                        tile_position=(0, sub * 64))
                                nc.vector.tensor_copy(
                                    xnT[p][:, qc * 128:(qc + 1) * 128],
                                    ptr[0:128, 0:128])

                        for h in range(HL):
                            scores_h(3, h, scp)
                            if h >= 1:
                                pv_h(h - 1)
                                if (h - 1) % 2 == 1:
                                    transp_p((h - 1) // 2)
                        pv_h(HL - 1)
                        transp_p(NPAIR - 1)

                if DEBUG:
                    nc.sync.dma_start(d_qpl[:], qpl[:])
                    nc.sync.dma_start(d_kpl[:], kpl[0][:])
                    nc.sync.dma_start(d_v65[:], v65[:])
                    nc.sync.dma_start(d_xn[:], xn[:])
                    nc.sync.dma_start(d_pt[:, 0:1024], pt_view(0, 0, 0))
                    nc.sync.dma_start(d_pt[:, 1024:2048], pt_view(3, 7, 1))
                # == out projection (all score/PV psum freed) ==
                with ExitStack() as tl:
                    pop = tl.enter_context(
                        tc.tile_pool(name="pop", bufs=3, space="PSUM"))
                    wp_ = wp_t[:].rearrange("p (a s) -> p a s", s=1024)
                    for j in range(8):
                        poq = pop.tile([128, 512], F32, tag="po", name="po")
                        for p in range(NPAIR):
                            nc.tensor.matmul(
                                poq[:],
                                wp_[:, p, j * 128:(j + 1) * 128],
                                xnT[p][:],
                                start=(p == 0), stop=(p == NPAIR - 1))
                        # osb space: reuse the dead kpl tiles (bitcast f32)
                        ob = kpl[j // 2][:].bitcast(F32)[
                            :, (j % 2) * 512:(j % 2 + 1) * 512]
                        if j % 2 == 0:
                            nc.scalar.copy(ob, poq[:])
                        else:
                            nc.vector.tensor_copy(ob, poq[:])
                        if j % 2 == 1:
                            obv = kpl[j // 2][:].bitcast(F32)[:, 0:1024]
                            nc.sync.dma_start(
                                outT[(j - 1) * 128:(j + 1) * 128,
                                     :].rearrange(
                                    "(a p) s -> p a s", p=128),
                                obv.rearrange("p (a s) -> p a s", s=512))

    nc.compile()
    return nc


def prep_inputs(query, key, value, qpos, kpos, Wq, Wk, Wv, Wp, bp):
    """Build per-core input maps (8 cores: core = 2*b + g)."""
    invf = (1.0 / ROPE_BASE ** (np.arange(0, D, 2, dtype=np.float32) / D)
            ).astype(np.float32)
    rows64 = invf[np.arange(64) % 32]          # [64]

    R64 = np.zeros((64, 64), dtype=np.float32)
    for r in range(32):
        R64[r, r + 32] = -1.0
        R64[r + 32, r] = 1.0
    rT128 = np.zeros((128, 128), dtype=np.float32)
    rT128[0:64, 0:64] = R64.T
    rT128[64:128, 64:128] = R64.T
    cst_np = np.concatenate(
        [rT128, np.eye(128, dtype=np.float32)], axis=1).astype(F16NP)

    def hilo(x):
        x = np.ascontiguousarray(x, dtype=np.float32)
        hi = x.astype(E4NP)
        lo = (x - hi.astype(np.float32)).astype(E4NP)
        return hi, lo

    qf = np.asarray(query, np.float32)
    kf = np.asarray(key, np.float32)
    vf = np.asarray(value, np.float32)
    q8 = {b: hilo(qf[b].T) for b in range(B)}
    k8 = {b: hilo(kf[b].T) for b in range(B)}
    v8 = {b: hilo(vf[b].T) for b in range(B)}

    in_maps = []
    for core in range(8):
        b, g = core // 2, core % 2
        cols = slice(g * CH, (g + 1) * CH)
        qang = rows64[:, None] * np.asarray(qpos[b], np.float32)[None, :]
        kang = rows64[:, None] * np.asarray(kpos[b], np.float32)[None, :]
        qtab_np = np.concatenate(
            [np.tile(np.cos(qang), (2, 1)), np.tile(np.sin(qang), (2, 1))],
            axis=1).astype(F16NP)
        ktab_np = np.concatenate(
            [np.concatenate(
                [np.tile(np.cos(kang[:, s * 512:(s + 1) * 512]), (2, 1)),
                 np.tile(np.sin(kang[:, s * 512:(s + 1) * 512]), (2, 1))],
                axis=1)
             for s in range(SB)], axis=1).astype(F16NP)
        m = {
            "qhT": q8[b][0], "qlT": q8[b][1],
            "khT": k8[b][0], "klT": k8[b][1],
            "vhT": v8[b][0], "vlT": v8[b][1],
            "wqT": np.ascontiguousarray(
                np.asarray(Wq, np.float32)[cols, :].T * WS).astype(E4NP),
            "wkT": np.ascontiguousarray(
                np.asarray(Wk, np.float32)[cols, :].T * WS).astype(E4NP),
            "wvT": np.ascontiguousarray(
                np.asarray(Wv, np.float32)[cols, :].T * WS).astype(E4NP),
            "wpT": np.ascontiguousarray(
                np.asarray(Wp, np.float32)[:, cols].T).astype(F16NP),
            "cst": cst_np,
            "qtab": qtab_np,
            "ktab": ktab_np,
        }
        in_maps.append(m)
    return in_maps


_NC_CACHE = {}


def _get_nc(iters=1):
    if iters not in _NC_CACHE:
        _NC_CACHE[iters] = build_nc(iters)
    return _NC_CACHE[iters]


def kernel(query, key, value, qpos, kpos, Wq, Wk, Wv, Wp, bp):
    from concourse.bass_utils import run_bass_kernel_spmd

    nc = _get_nc()
    in_maps = prep_inputs(query, key, value, qpos, kpos, Wq, Wk, Wv, Wp, bp)
    res = run_bass_kernel_spmd(nc, in_maps, list(range(8)))
    out = np.zeros((B, Nq, C), dtype=np.float32)
    for core in range(8):
        out[core // 2] += res.results[core]["outT"].T
    out += np.asarray(bp, np.float32)
    return out


# revision 25
# speedup vs baseline: 1.0886x; 1.0886x over previous
"""CrossAttention (RoPE, 16 heads, C=1024) Trainium2 Bass kernel, v3.

Sharding: DP over batch (4) x TP over heads (2 groups of 8) = 8 cores.

Numerics (error-budgeted against the 2e-2 gate):
- Q/K projections: fp8e4m3 DoubleRow with hi+lo input decomposition
  (2048 effective contraction rows), weights fp8*WS.
- RoPE via the frequency-symmetry identity: the k side needs NO rotation:
    S = sum_d kcos[d]*qpl0[d] + ksin[d]*qpl1[d]
  with kcos = k^*cos_k, ksin = k^*sin_k (plain muls straight from psum),
    qpl0 = q^*cos_q + rot(q^*sin_q),  qpl1 = q^*sin_q - rot(q^*cos_q).
  Only the 4 q-tiles take PE rotate matmuls.
- Scores: fp8 DR, 2 real planes (kcos|ksin stationary, qpl0|qpl1 moving).
  exp on ACT -> fp16 probs.
- V projection: fp8 DR with v hi+lo as extra contraction chunks, wv
  fp8*WS moving; psum evac * (1/WS) -> fp16 v65 (+ones col for rowsums).
- PV/out-proj: fp16 (fp8 probs would blow the error budget), late
  normalization, PE transposes.
"""

import sys

if "/opt/trn_rl_repo" not in sys.path:
    sys.path.insert(0, "/opt/trn_rl_repo")

import numpy as np
import ml_dtypes
from contextlib import ExitStack

import concourse.bass as bass
import concourse.tile as tile
from concourse import bacc, mybir

from concourse.dve_spec import Spec, Src0, C0, C1, C2, One, sq, lower
from concourse import dve_ops as _dve_ops
from concourse.dve_ops import DveOp


def _register_exp_ops():
    if "EXP_POLY_A" in _dve_ops._SUB_OPCODE_FOR_NAME:
        return (_dve_ops.CUSTOM_DVE_SPECS["EXP_POLY_A"],)


def _np32(x):
    return np.asarray(x, np.float32)


def _ref_exp_a(in0, in1, c0, c1, c2):
    w = _np32(_np32(in0) * np.float32(c0))
    t = _np32(w * np.float32(c1))
    t2 = _np32(t + np.float32(c2))
    w2 = _np32(w * w)
    u = _np32(w2 * t2)
    v = _np32(u + w)
    return _np32(v + np.float32(1.0))


def _ref_exp_b(in0, in1, c0, c1, c2):
    x = _np32(in0)
    for _ in range(4):
        x = _np32(x * x)
    return x


_w = Src0 * C0
_EXP_A_BODY = (sq(_w) * (_w * C1 + C2) + _w) + One
_EXP_B_BODY = sq(sq(sq(sq(Src0))))


def _ref_exp_c(in0, in1, c0, c1, c2):
    v = _np32(_np32(in0) * np.float32(c0) + np.float32(c1))
    t = _np32(v * v + np.float32(c2))
    for _ in range(4):
        t = _np32(t * t)
    return t


_EXP_C_BODY = sq(sq(sq(sq(sq(Src0 * C0 + C1) + C2))))

EXP_POLY_A = DveOp(
    "EXP_POLY_A",
    Spec(body=_EXP_A_BODY, reference=_ref_exp_a),
    subdim=False,
    uops_sha={},
)
EXP_POLY_B = DveOp(
    "EXP_POLY_B",
    Spec(body=_EXP_B_BODY, reference=_ref_exp_b),
    subdim=False,
    uops_sha={},
)
EXP_POLY_C = DveOp(
    "EXP_POLY_C",
    Spec(body=_EXP_C_BODY, reference=_ref_exp_c),
    subdim=False,
    uops_sha={},
)

for _op in (EXP_POLY_A, EXP_POLY_B, EXP_POLY_C):
    if _op.name not in _dve_ops._SUB_OPCODE_FOR_NAME:
        _dve_ops.OPS.append(_op)
        _dve_ops._SUB_OPCODE_FOR_NAME[_op.name] = (
            _dve_ops._CUSTOM_DVE_ROW_BASE + len(_dve_ops.OPS) - 1)
        _dve_ops.CUSTOM_DVE_SPECS[_op.name] = _op.spec
    for _ver in ("v3", "v4"):
        try:
            _op.compile(_ver)
        except ValueError as e:
            import re as _re
            _mm = _re.search(r'uops_sha\["' + _ver + r'"\]="([0-9a-f]+)"',
                             str(e))
            if not _mm:
                raise
            _op.uops_sha[_ver] = _mm.group(1)
            _op.compile(_ver)

F32 = mybir.dt.float32
F16 = mybir.dt.float16
F8 = mybir.dt.float8e4
DR = mybir.MatmulPerfMode.DoubleRow
EXP = mybir.ActivationFunctionType.Exp
E4NP = ml_dtypes.float8_e4m3
F16NP = np.float16

# problem constants
B, Nq, Nk, C = 4, 512, 2048, 1024
H, D = 16, 64
HL = 8            # heads per core
CH = HL * D       # 512 local channels
NPAIR = HL // 2   # 4 pairs of heads (128 rows each)
SB = Nk // 512    # 4 key blocks of 512
SC = Nk // 128    # 16 key chunks of 128
ROPE_BASE = 10000.0
SCALE = float(D) ** -0.5
WS = 32.0         # fp8 weight prescale (32 keeps qpl/kpl < e4m3 max)
ESC = SCALE / (WS * WS)   # folded into the exp activation scale
DEBUG = False
N_DVE_EXP = 10            # of 64 exp tiles, how many go to the DVE poly path


def _ld3(eng, dst_view, src_2d, width=512):
    """DMA a [N*128, width] DRAM region into a [128, N*width] tile view."""
    eng.dma_start(
        dst_view.rearrange("p (a s) -> p a s", s=width),
        src_2d.rearrange("(a p) s -> p a s", p=128))


def build_nc(iters: int = 1):
    nc = bacc.Bacc("TRN2", target_bir_lowering=False, debug=False)

    qhT = nc.dram_tensor("qhT", [C, Nq], F8, kind="ExternalInput")
    qlT = nc.dram_tensor("qlT", [C, Nq], F8, kind="ExternalInput")
    khT = nc.dram_tensor("khT", [C, Nk], F8, kind="ExternalInput")
    klT = nc.dram_tensor("klT", [C, Nk], F8, kind="ExternalInput")
    vhT = nc.dram_tensor("vhT", [C, Nk], F8, kind="ExternalInput")
    vlT = nc.dram_tensor("vlT", [C, Nk], F8, kind="ExternalInput")
    wqT = nc.dram_tensor("wqT", [C, CH], F8, kind="ExternalInput")
    wkT = nc.dram_tensor("wkT", [C, CH], F8, kind="ExternalInput")
    wvT = nc.dram_tensor("wvT", [C, CH], F8, kind="ExternalInput")
    wpT = nc.dram_tensor("wpT", [CH, C], F16, kind="ExternalInput")
    # cst = [rT | idn]; qtab = [cq | sq]; ktab = per-block [ck | sk]
    cst = nc.dram_tensor("cst", [128, 256], F16, kind="ExternalInput")
    qtab = nc.dram_tensor("qtab", [128, 2 * Nq], F16, kind="ExternalInput")
    ktab = nc.dram_tensor("ktab", [128, 2 * Nk], F16, kind="ExternalInput")
    outT = nc.dram_tensor("outT", [C, Nq], F32, kind="ExternalOutput")
    if DEBUG:
        d_qpl = nc.dram_tensor("d_qpl", [128, NPAIR * 1024], F8,
                               kind="ExternalOutput")
        d_kpl = nc.dram_tensor("d_kpl", [128, 2 * Nk], F8,
                               kind="ExternalOutput")
        d_v65 = nc.dram_tensor("d_v65", [128, SC * 520], F16,
                               kind="ExternalOutput")
        d_xn = nc.dram_tensor("d_xn", [128, HL * 4 * 64], F16,
                              kind="ExternalOutput")
        d_pt = nc.dram_tensor("d_pt", [128, 2048], F16,
                              kind="ExternalOutput")

    with tile.TileContext(nc) as tc, ExitStack() as top:
        const = top.enter_context(tc.tile_pool(name="const", bufs=1))
        cst_t = const.tile([128, 256], F16, tag="cst", name="cst")
        rt_t = cst_t[0:128, 0:128]
        id_t = cst_t[0:128, 128:256]
        cst_loaded = [False]

        for _ in range(iters):
            with ExitStack() as it:
                per = it.enter_context(tc.tile_pool(name="per", bufs=1))
                qpl = per.tile([128, NPAIR * 1024], F8, tag="qpl", name="qpl")
                kpl = [per.tile([128, 2 * Nk], F8, tag=f"kpl{m}",
                                name=f"kpl{m}") for m in range(NPAIR)]
                v65 = per.tile([128, SC * 520], F16, tag="v65", name="v65")
                inv_t = per.tile([128, HL * 4], F32, tag="inv", name="inv")
                xn = per.tile([128, HL * 4 * 64], F16, tag="xn", name="xn")
                # all 64 prob tiles live in one [128, 64K] fp16 tile;
                # (sbi, h, jj) -> columns [idx*1024, (idx+1)*1024)
                pts_t = per.tile([128, 64 * 1024], F16, tag="pts",
                                 name="pts")

                def pt_view(sbi, h, jj):
                    idx = sbi * 16 + h * 2 + jj
                    return pts_t[:, idx * 1024:(idx + 1) * 1024]

                stg = it.enter_context(ExitStack())
                ktm = stg.enter_context(tc.tile_pool(name="ktm", bufs=2))
                w8 = stg.enter_context(tc.tile_pool(name="w8", bufs=1))
                kst = stg.enter_context(tc.tile_pool(name="kst", bufs=1))
                ktb = stg.enter_context(tc.tile_pool(name="ktb", bufs=2))
                s0 = ExitStack()   # block-0-lifetime staging
                qst = s0.enter_context(tc.tile_pool(name="qst", bufs=1))
                qtb = s0.enter_context(tc.tile_pool(name="qtb", bufs=1))
                atm = s0.enter_context(tc.tile_pool(name="atm", bufs=2))
                wqp = s0.enter_context(tc.tile_pool(name="wqp", bufs=1))
                vs = ExitStack()   # v-projection-lifetime staging

                # ---- SP queue: k-side stream (gates pipeline start);
                #      ACT queue: q-side + v/p weights (ACT idle in lead) ----
                if not cst_loaded[0]:
                    nc.scalar.dma_start(cst_t[:], cst[:])
                    cst_loaded[0] = True
                wk_t = w8.tile([128, 8 * CH], F8, tag="w8", name="wk")
                _ld3(nc.sync, wk_t[:], wkT[:], CH)
                wq_t = wqp.tile([128, 8 * CH], F8, tag="wq", name="wq")
                _ld3(nc.scalar, wq_t[:], wqT[:], CH)
                qt_t = qst.tile([128, 2 * 4096], F8, tag="qt", name="qt")
                _ld3(nc.scalar, qt_t[:, 0:4096], qhT[:], 512)
                _ld3(nc.scalar, qt_t[:, 4096:8192], qlT[:], 512)
                qtab_t = qtb.tile([128, 2 * Nq], F16, tag="qtb", name="qtab")
                nc.scalar.dma_start(qtab_t[:], qtab[:])

                # ---- k/v block loads on the SP queue ----
                kt_ts, vt_ts, ktab_ts = [], [], []
                for sbi in range(SB):
                    sl = slice(sbi * 512, (sbi + 1) * 512)
                    kt_t = kst.tile([128, 8192], F8, tag="kt", name="kt")
                    _ld3(nc.sync, kt_t[:, 0:4096], khT[:, sl], 512)
                    _ld3(nc.sync, kt_t[:, 4096:8192], klT[:, sl], 512)
                    ktab_t = ktb.tile([128, 1024], F16, tag="ktb",
                                      name="ktab")
                    nc.sync.dma_start(
                        ktab_t[:], ktab[:, sbi * 1024:(sbi + 1) * 1024])
                    kt_ts.append(kt_t)
                    ktab_ts.append(ktab_t)

                def proj_hilo(pq, w_t, x_t, m, width):
                    wv_ = w_t[:].rearrange("p (a m) -> p a m", m=CH)
                    xv_ = x_t[:].rearrange("p (a n) -> p a n", n=width)
                    for s in range(8):
                        ws = s % 4
                        nc.tensor.matmul(
                            pq[:],
                            wv_[:, 2 * ws:2 * ws + 2, m * 128:(m + 1) * 128],
                            xv_[:, 2 * s:2 * s + 2, :],
                            start=(s == 0), stop=(s == 7), perf_mode=DR)

                _pk_live = {}

                def kproj_half(sbi, m, psum_pool, half):
                    wv_ = wk_t[:].rearrange("p (a m) -> p a m", m=CH)
                    xv_ = kt_ts[sbi][:].rearrange("p (a n) -> p a n", n=512)
                    if half == 0:
                        _pk_live[(sbi, m)] = psum_pool.tile(
                            [128, 512], F32, tag="pk", name="pk")
                    pk = _pk_live[(sbi, m)]
                    for s in range(4 * half, 4 * half + 4):
                        ws = s % 4
                        nc.tensor.matmul(
                            pk[:],
                            wv_[:, 2 * ws:2 * ws + 2, m * 128:(m + 1) * 128],
                            xv_[:, 2 * s:2 * s + 2, :],
                            start=(s == 0), stop=(s == 7), perf_mode=DR)
                    if half == 1:
                        ktab_t = ktab_ts[sbi]
                        xk = ktm.tile([128, 512], F16, tag="xk", name="xk")
                        nc.vector.tensor_copy(xk[:], pk[:])
                        nc.gpsimd.tensor_mul(
                            kpl[m][:, sbi * 512:(sbi + 1) * 512],
                            xk[:], ktab_t[0:128, 0:512])
                        nc.gpsimd.tensor_mul(
                            kpl[m][:, Nk + sbi * 512:Nk + (sbi + 1) * 512],
                            xk[:], ktab_t[0:128, 512:1024])

                def kproj_m(sbi, m, psum_pool):
                    pk = psum_pool.tile([128, 512], F32, tag="pk", name="pk")
                    proj_hilo(pk, wk_t, kt_ts[sbi], m, 512)
                    ktab_t = ktab_ts[sbi]
                    xk = ktm.tile([128, 512], F16, tag="xk", name="xk")
                    nc.vector.tensor_copy(xk[:], pk[:])
                    nc.gpsimd.tensor_mul(
                        kpl[m][:, sbi * 512:(sbi + 1) * 512],
                        xk[:], ktab_t[0:128, 0:512])
                    nc.gpsimd.tensor_mul(
                        kpl[m][:, Nk + sbi * 512:Nk + (sbi + 1) * 512],
                        xk[:], ktab_t[0:128, 512:1024])

                def qside_m(m, psum_pool, rot_pool):
                    pq = psum_pool.tile([128, 512], F32, tag="pk", name="pq")
                    proj_hilo(pq, wq_t, qt_t, m, Nq)
                    xsq = atm.tile([128, Nq], F16, tag="xsq", name="xsq")
                    nc.scalar.copy(xsq[:], pq[:])
                    m_c = atm.tile([128, Nq], F16, tag="mc", name="mc")
                    nc.gpsimd.tensor_mul(m_c[:], xsq[:], qtab_t[0:128, 0:Nq])
                    m_s = atm.tile([128, Nq], F16, tag="ms", name="ms")
                    nc.gpsimd.tensor_mul(m_s[:], xsq[:],
                                         qtab_t[0:128, Nq:2 * Nq])
                    prot0 = rot_pool.tile([128, Nq], F32, tag="prot",
                                          name="prot0")
                    nc.tensor.matmul(prot0[:], rt_t, m_s[:],
                                     start=True, stop=True)
                    nc.vector.tensor_add(
                        qpl[:, m * 1024:m * 1024 + 512], m_c[:], prot0[:])
                    prot1 = rot_pool.tile([128, Nq], F32, tag="prot",
                                          name="prot1")
                    nc.tensor.matmul(prot1[:], rt_t, m_c[:],
                                     start=True, stop=True)
                    nc.vector.tensor_sub(
                        qpl[:, m * 1024 + 512:(m + 1) * 1024],
                        m_s[:], prot1[:])

                def vproj_chunk(vb, scj, pvp):
                    vt_ = vt_ts[vb][:].rearrange("p (a n) -> p a n", n=512)
                    wv_ = wv_t[:].rearrange("p (a m) -> p a m", m=CH)
                    sc = vb * 4 + scj
                    pv = pvp.tile([128, CH], F32, tag="pv", name="pv")
                    for s in range(8):
                        ws = s % 4
                        nc.tensor.matmul(
                            pv[:],
                            vt_[:, 2 * s:2 * s + 2,
                                scj * 128:(scj + 1) * 128],
                            wv_[:, 2 * ws:2 * ws + 2, :],
                            start=(s == 0), stop=(s == 7), perf_mode=DR)
                    nc.vector.tensor_scalar_mul(
                        v65[:, sc * 520:(sc + 1) * 520
                            ].rearrange("p (n w) -> p n w",
                                        w=65)[:, :, 0:64],
                        pv[:].rearrange("p (n w) -> p n w", w=64),
                        1.0 / WS)

                def scores_h(sbi, h, scp):
                    m, r0 = h // 2, 64 * (h % 2)
                    stv = kpl[m][r0:r0 + 64, :].rearrange(
                        "p (two n) -> p two n", two=2)
                    mvv = qpl[r0:r0 + 64,
                              m * 1024:(m + 1) * 1024].rearrange(
                        "p (two n) -> p two n", two=2)
                    for jj in range(2):
                        psc = scp.tile([128, 1024], F32, tag="psc",
                                       name="psc")
                        for j2 in range(2):
                            sc = sbi * 4 + jj * 2 + j2
                            nc.tensor.matmul(
                                psc[:, j2 * 512:(j2 + 1) * 512],
                                stv[:, :, sc * 128:(sc + 1) * 128],
                                mvv, start=True, stop=True, perf_mode=DR)
                        pt = pt_view(sbi, h, jj)
                        idx = sbi * 16 + h * 2 + jj
                        if N_DVE_EXP > 0 and (
                                (idx < 48 and idx % 7 in (1, 4))
                                or idx in (49, 52, 55, 58)):
                            nc.vector._custom_dve(
                                EXP_POLY_C, out=pt, in0=psc[:],
                                s0=float(ESC / 16.0 * 0.5 ** 0.5),
                                s1=float(0.5 ** 0.5), imm2=0.5)
                        else:
                            nc.scalar.activation(pt, psc[:], EXP, scale=ESC)
                        pts[(sbi, h, jj)] = pt

                pts = {}
                with ExitStack() as phb:
                    scp = phb.enter_context(
                        tc.tile_pool(name="scp", bufs=2, space="PSUM"))
                    pps_stack = ExitStack()
                    pps = pps_stack.enter_context(
                        tc.tile_pool(name="pps", bufs=2, space="PSUM"))
                    # block 0: per-pair weave; block-1 k-proj rides along
                    with ExitStack() as ph0:
                        rp0 = ph0.enter_context(
                            tc.tile_pool(name="rp0", bufs=2, space="PSUM"))
                        for m in range(NPAIR):
                            kproj_m(0, m, pps)
                            qside_m(m, pps, rp0)
                            scores_h(0, 2 * m, scp)
                            scores_h(0, 2 * m + 1, scp)
                    s0.close()
                    nc.vector.memset(
                        v65[:].rearrange("p (s h w) -> p s h w", h=HL,
                                         w=65)[:, :, :, 64:65], 1.0)
                    wvp = vs.enter_context(tc.tile_pool(name="wvp", bufs=1))
                    vst = vs.enter_context(tc.tile_pool(name="vst", bufs=2))
                    wv_t = wvp.tile([128, 8 * CH], F8, tag="wv", name="wv")
                    _ld3(nc.sync, wv_t[:], wvT[:], CH)
                    for _sbi in range(SB):
                        _sl = slice(_sbi * 512, (_sbi + 1) * 512)
                        vt_t = vst.tile([128, 8192], F8, tag="vt", name="vt")
                        _ld3(nc.sync, vt_t[:, 0:4096], vhT[:, _sl], 512)
                        _ld3(nc.sync, vt_t[:, 4096:8192], vlT[:, _sl], 512)
                        vt_ts.append(vt_t)
                    # blocks 1-2: scores woven with next-block k-proj and
                    # the v projections (thunk queue, ~1-2 per head)
                    with ExitStack() as phk:
                        pvp = phk.enter_context(
                            tc.tile_pool(name="pvp", bufs=2, space="PSUM"))
                        for m in range(NPAIR):
                            kproj_m(1, m, pps)
                        weaves = {
                            1: ([lambda m=q // 2, hf=q % 2:
                                 kproj_half(2, m, pps, hf)
                                 for q in range(2 * NPAIR)]
                                + [lambda c=c: vproj_chunk(c // 4, c % 4,
                                                           pvp)
                                   for c in range(0, 8)]),
                            2: ([lambda c=c: vproj_chunk(c // 4, c % 4,
                                                         pvp)
                                 for c in range(8, 12)]
                                + [lambda q=q: kproj_half(3, q // 2, pps,
                                                          q % 2)
                                   for q in range(2 * NPAIR)]
                                + [lambda c=c: vproj_chunk(c // 4, c % 4,
                                                           pvp)
                                   for c in range(12, 16)]),
                        }
                        for sbi in (1, 2):
                            wq_ = weaves[sbi]
                            nper = (len(wq_) + HL - 1) // HL
                            for h in range(HL):
                                scores_h(sbi, h, scp)
                                for t in wq_[h * nper:(h + 1) * nper]:
                                    t()
                    vs.close()
                    stg.close()
                    pps_stack.close()
                    wpp = it.enter_context(tc.tile_pool(name="wpp", bufs=1))
                    wp_t = wpp.tile([128, NPAIR * 1024], F16, tag="wp",
                                    name="wp")
                    _ld3(nc.sync, wp_t[:], wpT[:], 1024)

                    # block-3 scores with PV/normalization/transposes woven
                    # in one head behind the exp wave
                    xtt = it.enter_context(
                        tc.tile_pool(name="xtt", bufs=1))
                    with ExitStack() as phx:
                        xtp = phx.enter_context(
                            tc.tile_pool(name="xtp", bufs=2, space="PSUM"))
                        xnT = [xtt.tile([128, Nq], F16, tag=f"xnT{p}",
                                        name=f"xnT{p}")
                               for p in range(NPAIR)]

                        def pv_h(h):
                            pxt = xtp.tile([128, 260], F32, tag="pxt",
                                           name="pxt",
                                           padded_shape=[128, 512])
                            for qc in range(4):
                                for sc in range(SC):
                                    sbi, jj, j2 = (sc // 4, (sc % 4) // 2,
                                                   sc % 2)
                                    nc.tensor.matmul(
                                        pxt[:, qc * 65:(qc + 1) * 65],
                                        pts[(sbi, h, jj)][
                                            :, j2 * 512 + qc * 128:
                                            j2 * 512 + (qc + 1) * 128],
                                        v65[:, sc * 520 + h * 65:
                                            sc * 520 + (h + 1) * 65],
                                        start=(sc == 0), stop=(sc == SC - 1))
                            nc.vector.reciprocal(
                                inv_t[:, h * 4:(h + 1) * 4].rearrange(
                                    "p (a b) -> p a b", b=1),
                                pxt[:].rearrange("p (q w) -> p q w",
                                                 w=65)[:, :, 64:65])
                            for qc in range(4):
                                nc.vector.tensor_scalar_mul(
                                    xn[:, (h * 4 + qc) * 64:
                                       (h * 4 + qc + 1) * 64],
                                    pxt[:, qc * 65:qc * 65 + 64],
                                    inv_t[:, h * 4 + qc:h * 4 + qc + 1])

                        def transp_p(p):
                            for qc in range(4):
                                ptf = xtp.tile([128, 256], F32, tag="pxt",
                                               name="ptr",
                                               padded_shape=[128, 512])
                                ptr = ptf[:].bitcast(F16)
                                for sub in range(2):
                                    hh = 2 * p + sub
                                    nc.tensor.transpose(
                                        ptr[sub * 64:(sub + 1) * 64, 0:128],
                                        xn[:, (hh * 4 + qc) * 64:
                                           (hh * 4 + qc + 1) * 64],
                                        id_t[:],
                                        tile_position=(0, sub * 64))
                                nc.vector.tensor_copy(
                                    xnT[p][:, qc * 128:(qc + 1) * 128],
                                    ptr[0:128, 0:128])

                        for h in range(HL):
                            scores_h(3, h, scp)
                            if h >= 1:
                                pv_h(h - 1)
                                if (h - 1) % 2 == 1:
                                    transp_p((h - 1) // 2)
                        pv_h(HL - 1)
                        transp_p(NPAIR - 1)

                if DEBUG:
                    nc.sync.dma_start(d_qpl[:], qpl[:])
                    nc.sync.dma_start(d_kpl[:], kpl[0][:])
                    nc.sync.dma_start(d_v65[:], v65[:])
                    nc.sync.dma_start(d_xn[:], xn[:])
                    nc.sync.dma_start(d_pt[:, 0:1024], pt_view(0, 0, 0))
                    nc.sync.dma_start(d_pt[:, 1024:2048], pt_view(3, 7, 1))
                # == out projection (all score/PV psum freed) ==
                with ExitStack() as tl:
                    pop = tl.enter_context(
                        tc.tile_pool(name="pop", bufs=3, space="PSUM"))
                    wp_ = wp_t[:].rearrange("p (a s) -> p a s", s=1024)
                    for j in range(8):
                        poq = pop.tile([128, 512], F32, tag="po", name="po")
                        for p in range(NPAIR):
                            nc.tensor.matmul(
                                poq[:],
                                wp_[:, p, j * 128:(j + 1) * 128],
                                xnT[p][:],
                                start=(p == 0), stop=(p == NPAIR - 1))
                        # osb space: reuse the dead kpl tiles (bitcast f32)
                        ob = kpl[j // 2][:].bitcast(F32)[
                            :, (j % 2) * 512:(j % 2 + 1) * 512]
                        if j % 2 == 0:
                            nc.scalar.copy(ob, poq[:])
                        else:
                            nc.vector.tensor_copy(ob, poq[:])
                        if j % 2 == 1:
                            obv = kpl[j // 2][:].bitcast(F32)[:, 0:1024]
                            nc.sync.dma_start(
                                outT[(j - 1) * 128:(j + 1) * 128,
                                     :].rearrange(
                                    "(a p) s -> p a s", p=128),
                                obv.rearrange("p (a s) -> p a s", s=512))

    nc.compile()
    return nc


def prep_inputs(query, key, value, qpos, kpos, Wq, Wk, Wv, Wp, bp):
    """Build per-core input maps (8 cores: core = 2*b + g)."""
    invf = (1.0 / ROPE_BASE ** (np.arange(0, D, 2, dtype=np.float32) / D)
            ).astype(np.float32)
    rows64 = invf[np.arange(64) % 32]          # [64]

    R64 = np.zeros((64, 64), dtype=np.float32)
    for r in range(32):
        R64[r, r + 32] = -1.0
        R64[r + 32, r] = 1.0
    rT128 = np.zeros((128, 128), dtype=np.float32)
    rT128[0:64, 0:64] = R64.T
    rT128[64:128, 64:128] = R64.T
    cst_np = np.concatenate(
        [rT128, np.eye(128, dtype=np.float32)], axis=1).astype(F16NP)

    def hilo(x):
        x = np.ascontiguousarray(x, dtype=np.float32)
        hi = x.astype(E4NP)
        lo = (x - hi.astype(np.float32)).astype(E4NP)
        return hi, lo

    qf = np.asarray(query, np.float32)
    kf = np.asarray(key, np.float32)
    vf = np.asarray(value, np.float32)
    q8 = {b: hilo(qf[b].T) for b in range(B)}
    k8 = {b: hilo(kf[b].T) for b in range(B)}
    v8 = {b: hilo(vf[b].T) for b in range(B)}

    in_maps = []
    for core in range(8):
        b, g = core // 2, core % 2
        cols = slice(g * CH, (g + 1) * CH)
        qang = rows64[:, None] * np.asarray(qpos[b], np.float32)[None, :]
        kang = rows64[:, None] * np.asarray(kpos[b], np.float32)[None, :]
        qtab_np = np.concatenate(
            [np.tile(np.cos(qang), (2, 1)), np.tile(np.sin(qang), (2, 1))],
            axis=1).astype(F16NP)
        ktab_np = np.concatenate(
            [np.concatenate(
                [np.tile(np.cos(kang[:, s * 512:(s + 1) * 512]), (2, 1)),
                 np.tile(np.sin(kang[:, s * 512:(s + 1) * 512]), (2, 1))],
                axis=1)
             for s in range(SB)], axis=1).astype(F16NP)
        m = {
            "qhT": q8[b][0], "qlT": q8[b][1],
            "khT": k8[b][0], "klT": k8[b][1],
            "vhT": v8[b][0], "vlT": v8[b][1],
            "wqT": np.ascontiguousarray(
                np.asarray(Wq, np.float32)[cols, :].T * WS).astype(E4NP),
            "wkT": np.ascontiguousarray(
                np.asarray(Wk, np.float32)[cols, :].T * WS).astype(E4NP),
            "wvT": np.ascontiguousarray(
                np.asarray(Wv, np.float32)[cols, :].T * WS).astype(E4NP),
            "wpT": np.ascontiguousarray(
                np.asarray(Wp, np.float32)[:, cols].T).astype(F16NP),
            "cst": cst_np,
            "qtab": qtab_np,
            "ktab": ktab_np,
        }
        in_maps.append(m)
    return in_maps


_NC_CACHE = {}


def _get_nc(iters=1):
    if iters not in _NC_CACHE:
        _NC_CACHE[iters] = build_nc(iters)
    return _NC_CACHE[iters]


def kernel(query, key, value, qpos, kpos, Wq, Wk, Wv, Wp, bp):
    from concourse.bass_utils import run_bass_kernel_spmd

    nc = _get_nc()
    in_maps = prep_inputs(query, key, value, qpos, kpos, Wq, Wk, Wv, Wp, bp)
    res = run_bass_kernel_spmd(nc, in_maps, list(range(8)))
    out = np.zeros((B, Nq, C), dtype=np.float32)
    for core in range(8):
        out[core // 2] += res.results[core]["outT"].T
    out += np.asarray(bp, np.float32)
    return out


# revision 26
# speedup vs baseline: 1.1105x; 1.0201x over previous
"""CrossAttention (RoPE, 16 heads, C=1024) Trainium2 Bass kernel, v3.

Sharding: DP over batch (4) x TP over heads (2 groups of 8) = 8 cores.

Numerics (error-budgeted against the 2e-2 gate):
- Q/K projections: fp8e4m3 DoubleRow with hi+lo input decomposition
  (2048 effective contraction rows), weights fp8*WS.
- RoPE via the frequency-symmetry identity: the k side needs NO rotation:
    S = sum_d kcos[d]*qpl0[d] + ksin[d]*qpl1[d]
  with kcos = k^*cos_k, ksin = k^*sin_k (plain muls straight from psum),
    qpl0 = q^*cos_q + rot(q^*sin_q),  qpl1 = q^*sin_q - rot(q^*cos_q).
  Only the 4 q-tiles take PE rotate matmuls.
- Scores: fp8 DR, 2 real planes (kcos|ksin stationary, qpl0|qpl1 moving).
  exp on ACT -> fp16 probs.
- V projection: fp8 DR with v hi+lo as extra contraction chunks, wv
  fp8*WS moving; psum evac * (1/WS) -> fp16 v65 (+ones col for rowsums).
- PV/out-proj: fp16 (fp8 probs would blow the error budget), late
  normalization, PE transposes.
"""

import sys

if "/opt/trn_rl_repo" not in sys.path:
    sys.path.insert(0, "/opt/trn_rl_repo")

import numpy as np
import ml_dtypes
from contextlib import ExitStack

import concourse.bass as bass
import concourse.tile as tile
from concourse import bacc, mybir

from concourse.dve_spec import Spec, Src0, C0, C1, C2, One, sq, lower
from concourse import dve_ops as _dve_ops
from concourse.dve_ops import DveOp


def _register_exp_ops():
    if "EXP_POLY_A" in _dve_ops._SUB_OPCODE_FOR_NAME:
        return (_dve_ops.CUSTOM_DVE_SPECS["EXP_POLY_A"],)


def _np32(x):
    return np.asarray(x, np.float32)


def _ref_exp_a(in0, in1, c0, c1, c2):
    w = _np32(_np32(in0) * np.float32(c0))
    t = _np32(w * np.float32(c1))
    t2 = _np32(t + np.float32(c2))
    w2 = _np32(w * w)
    u = _np32(w2 * t2)
    v = _np32(u + w)
    return _np32(v + np.float32(1.0))


def _ref_exp_b(in0, in1, c0, c1, c2):
    x = _np32(in0)
    for _ in range(4):
        x = _np32(x * x)
    return x


_w = Src0 * C0
_EXP_A_BODY = (sq(_w) * (_w * C1 + C2) + _w) + One
_EXP_B_BODY = sq(sq(sq(sq(Src0))))


def _ref_exp_c(in0, in1, c0, c1, c2):
    v = _np32(_np32(in0) * np.float32(c0) + np.float32(c1))
    t = _np32(v * v + np.float32(c2))
    for _ in range(4):
        t = _np32(t * t)
    return t


_EXP_C_BODY = sq(sq(sq(sq(sq(Src0 * C0 + C1) + C2))))

EXP_POLY_A = DveOp(
    "EXP_POLY_A",
    Spec(body=_EXP_A_BODY, reference=_ref_exp_a),
    subdim=False,
    uops_sha={},
)
EXP_POLY_B = DveOp(
    "EXP_POLY_B",
    Spec(body=_EXP_B_BODY, reference=_ref_exp_b),
    subdim=False,
    uops_sha={},
)
EXP_POLY_C = DveOp(
    "EXP_POLY_C",
    Spec(body=_EXP_C_BODY, reference=_ref_exp_c),
    subdim=False,
    uops_sha={},
)

for _op in (EXP_POLY_A, EXP_POLY_B, EXP_POLY_C):
    if _op.name not in _dve_ops._SUB_OPCODE_FOR_NAME:
        _dve_ops.OPS.append(_op)
        _dve_ops._SUB_OPCODE_FOR_NAME[_op.name] = (
            _dve_ops._CUSTOM_DVE_ROW_BASE + len(_dve_ops.OPS) - 1)
        _dve_ops.CUSTOM_DVE_SPECS[_op.name] = _op.spec
    for _ver in ("v3", "v4"):
        try:
            _op.compile(_ver)
        except ValueError as e:
            import re as _re
            _mm = _re.search(r'uops_sha\["' + _ver + r'"\]="([0-9a-f]+)"',
                             str(e))
            if not _mm:
                raise
            _op.uops_sha[_ver] = _mm.group(1)
            _op.compile(_ver)

F32 = mybir.dt.float32
F16 = mybir.dt.float16
F8 = mybir.dt.float8e4
DR = mybir.MatmulPerfMode.DoubleRow
EXP = mybir.ActivationFunctionType.Exp
E4NP = ml_dtypes.float8_e4m3
F16NP = np.float16

# problem constants
B, Nq, Nk, C = 4, 512, 2048, 1024
H, D = 16, 64
HL = 8            # heads per core
CH = HL * D       # 512 local channels
NPAIR = HL // 2   # 4 pairs of heads (128 rows each)
SB = Nk // 512    # 4 key blocks of 512
SC = Nk // 128    # 16 key chunks of 128
ROPE_BASE = 10000.0
SCALE = float(D) ** -0.5
WS = 32.0         # fp8 weight prescale (32 keeps qpl/kpl < e4m3 max)
ESC = SCALE / (WS * WS)   # folded into the exp activation scale
DEBUG = False
N_DVE_EXP = 10            # of 64 exp tiles, how many go to the DVE poly path


def _ld3(eng, dst_view, src_2d, width=512):
    """DMA a [N*128, width] DRAM region into a [128, N*width] tile view."""
    eng.dma_start(
        dst_view.rearrange("p (a s) -> p a s", s=width),
        src_2d.rearrange("(a p) s -> p a s", p=128))


def build_nc(iters: int = 1):
    nc = bacc.Bacc("TRN2", target_bir_lowering=False, debug=False)

    qhT = nc.dram_tensor("qhT", [C, Nq], F8, kind="ExternalInput")
    qlT = nc.dram_tensor("qlT", [C, Nq], F8, kind="ExternalInput")
    khT = nc.dram_tensor("khT", [C, Nk], F8, kind="ExternalInput")
    vhT = nc.dram_tensor("vhT", [C, Nk], F8, kind="ExternalInput")
    vlT = nc.dram_tensor("vlT", [C, Nk], F8, kind="ExternalInput")
    wqT = nc.dram_tensor("wqT", [C, CH], F8, kind="ExternalInput")
    wkT = nc.dram_tensor("wkT", [C, CH], F8, kind="ExternalInput")
    wvT = nc.dram_tensor("wvT", [C, CH], F8, kind="ExternalInput")
    wpT = nc.dram_tensor("wpT", [CH, C], F16, kind="ExternalInput")
    # cst = [rT | idn]; qtab = [cq | sq]; ktab = per-block [ck | sk]
    cst = nc.dram_tensor("cst", [128, 256], F16, kind="ExternalInput")
    qtab = nc.dram_tensor("qtab", [128, 2 * Nq], F16, kind="ExternalInput")
    ktab = nc.dram_tensor("ktab", [128, 2 * Nk], F16, kind="ExternalInput")
    outT = nc.dram_tensor("outT", [C, Nq], F32, kind="ExternalOutput")
    if DEBUG:
        d_qpl = nc.dram_tensor("d_qpl", [128, NPAIR * 1024], F8,
                               kind="ExternalOutput")
        d_kpl = nc.dram_tensor("d_kpl", [128, 2 * Nk], F8,
                               kind="ExternalOutput")
        d_v65 = nc.dram_tensor("d_v65", [128, SC * 520], F16,
                               kind="ExternalOutput")
        d_xn = nc.dram_tensor("d_xn", [128, HL * 4 * 64], F16,
                              kind="ExternalOutput")
        d_pt = nc.dram_tensor("d_pt", [128, 2048], F16,
                              kind="ExternalOutput")

    with tile.TileContext(nc) as tc, ExitStack() as top:
        const = top.enter_context(tc.tile_pool(name="const", bufs=1))
        cst_t = const.tile([128, 256], F16, tag="cst", name="cst")
        rt_t = cst_t[0:128, 0:128]
        id_t = cst_t[0:128, 128:256]
        cst_loaded = [False]

        for _ in range(iters):
            with ExitStack() as it:
                per = it.enter_context(tc.tile_pool(name="per", bufs=1))
                qpl = per.tile([128, NPAIR * 1024], F8, tag="qpl", name="qpl")
                kpl = [per.tile([128, 2 * Nk], F8, tag=f"kpl{m}",
                                name=f"kpl{m}") for m in range(NPAIR)]
                v65 = per.tile([128, SC * 520], F16, tag="v65", name="v65")
                inv_t = per.tile([128, HL * 4], F32, tag="inv", name="inv")
                xn = per.tile([128, HL * 4 * 64], F16, tag="xn", name="xn")
                # all 64 prob tiles live in one [128, 64K] fp16 tile;
                # (sbi, h, jj) -> columns [idx*1024, (idx+1)*1024)
                pts_t = per.tile([128, 64 * 1024], F16, tag="pts",
                                 name="pts")

                def pt_view(sbi, h, jj):
                    idx = sbi * 16 + h * 2 + jj
                    return pts_t[:, idx * 1024:(idx + 1) * 1024]

                stg = it.enter_context(ExitStack())
                ktm = stg.enter_context(tc.tile_pool(name="ktm", bufs=2))
                w8 = stg.enter_context(tc.tile_pool(name="w8", bufs=1))
                kst = stg.enter_context(tc.tile_pool(name="kst", bufs=1))
                ktb = stg.enter_context(tc.tile_pool(name="ktb", bufs=2))
                s0 = ExitStack()   # block-0-lifetime staging
                qst = s0.enter_context(tc.tile_pool(name="qst", bufs=1))
                qtb = s0.enter_context(tc.tile_pool(name="qtb", bufs=1))
                atm = s0.enter_context(tc.tile_pool(name="atm", bufs=2))
                wqp = s0.enter_context(tc.tile_pool(name="wqp", bufs=1))
                vs = ExitStack()   # v-projection-lifetime staging

                # ---- SP queue: k-side stream (gates pipeline start);
                #      ACT queue: q-side + v/p weights (ACT idle in lead) ----
                if not cst_loaded[0]:
                    nc.scalar.dma_start(cst_t[:], cst[:])
                    cst_loaded[0] = True
                wk_t = w8.tile([128, 8 * CH], F8, tag="w8", name="wk")
                _ld3(nc.sync, wk_t[:], wkT[:], CH)
                wq_t = wqp.tile([128, 8 * CH], F8, tag="wq", name="wq")
                _ld3(nc.scalar, wq_t[:], wqT[:], CH)
                qt_t = qst.tile([128, 2 * 4096], F8, tag="qt", name="qt")
                _ld3(nc.scalar, qt_t[:, 0:4096], qhT[:], 512)
                _ld3(nc.scalar, qt_t[:, 4096:8192], qlT[:], 512)
                qtab_t = qtb.tile([128, 2 * Nq], F16, tag="qtb", name="qtab")
                nc.scalar.dma_start(qtab_t[:], qtab[:])

                # ---- k/v block loads on the SP queue ----
                kt_ts, vt_ts, ktab_ts = [], [], []
                for sbi in range(SB):
                    sl = slice(sbi * 512, (sbi + 1) * 512)
                    kt_t = kst.tile([128, 4096], F8, tag="kt", name="kt")
                    _ld3(nc.sync, kt_t[:], khT[:, sl], 512)
                    ktab_t = ktb.tile([128, 1024], F16, tag="ktb",
                                      name="ktab")
                    nc.sync.dma_start(
                        ktab_t[:], ktab[:, sbi * 1024:(sbi + 1) * 1024])
                    kt_ts.append(kt_t)
                    ktab_ts.append(ktab_t)

                def proj_hilo(pq, w_t, x_t, m, width):
                    wv_ = w_t[:].rearrange("p (a m) -> p a m", m=CH)
                    xv_ = x_t[:].rearrange("p (a n) -> p a n", n=width)
                    for s in range(8):
                        ws = s % 4
                        nc.tensor.matmul(
                            pq[:],
                            wv_[:, 2 * ws:2 * ws + 2, m * 128:(m + 1) * 128],
                            xv_[:, 2 * s:2 * s + 2, :],
                            start=(s == 0), stop=(s == 7), perf_mode=DR)

                _pk_live = {}

                def kproj_half(sbi, m, psum_pool, half):
                    wv_ = wk_t[:].rearrange("p (a m) -> p a m", m=CH)
                    xv_ = kt_ts[sbi][:].rearrange("p (a n) -> p a n", n=512)
                    if half == 0:
                        _pk_live[(sbi, m)] = psum_pool.tile(
                            [128, 512], F32, tag="pk", name="pk")
                    pk = _pk_live[(sbi, m)]
                    for s in range(2 * half, 2 * half + 2):
                        ws = s
                        nc.tensor.matmul(
                            pk[:],
                            wv_[:, 2 * ws:2 * ws + 2, m * 128:(m + 1) * 128],
                            xv_[:, 2 * s:2 * s + 2, :],
                            start=(s == 0), stop=(s == 3), perf_mode=DR)
                    if half == 1:
                        ktab_t = ktab_ts[sbi]
                        xk = ktm.tile([128, 512], F16, tag="xk", name="xk")
                        nc.vector.tensor_copy(xk[:], pk[:])
                        nc.gpsimd.tensor_mul(
                            kpl[m][:, sbi * 512:(sbi + 1) * 512],
                            xk[:], ktab_t[0:128, 0:512])
                        nc.gpsimd.tensor_mul(
                            kpl[m][:, Nk + sbi * 512:Nk + (sbi + 1) * 512],
                            xk[:], ktab_t[0:128, 512:1024])

                def kproj_m(sbi, m, psum_pool):
                    pk = psum_pool.tile([128, 512], F32, tag="pk", name="pk")
                    wv_ = wk_t[:].rearrange("p (a m) -> p a m", m=CH)
                    xv_ = kt_ts[sbi][:].rearrange("p (a n) -> p a n", n=512)
                    for s in range(4):
                        nc.tensor.matmul(
                            pk[:],
                            wv_[:, 2 * s:2 * s + 2, m * 128:(m + 1) * 128],
                            xv_[:, 2 * s:2 * s + 2, :],
                            start=(s == 0), stop=(s == 3), perf_mode=DR)
                    ktab_t = ktab_ts[sbi]
                    xk = ktm.tile([128, 512], F16, tag="xk", name="xk")
                    nc.vector.tensor_copy(xk[:], pk[:])
                    nc.gpsimd.tensor_mul(
                        kpl[m][:, sbi * 512:(sbi + 1) * 512],
                        xk[:], ktab_t[0:128, 0:512])
                    nc.gpsimd.tensor_mul(
                        kpl[m][:, Nk + sbi * 512:Nk + (sbi + 1) * 512],
                        xk[:], ktab_t[0:128, 512:1024])

                def qside_m(m, psum_pool, rot_pool):
                    pq = psum_pool.tile([128, 512], F32, tag="pk", name="pq")
                    proj_hilo(pq, wq_t, qt_t, m, Nq)
                    xsq = atm.tile([128, Nq], F16, tag="xsq", name="xsq")
                    nc.scalar.copy(xsq[:], pq[:])
                    m_c = atm.tile([128, Nq], F16, tag="mc", name="mc")
                    nc.gpsimd.tensor_mul(m_c[:], xsq[:], qtab_t[0:128, 0:Nq])
                    m_s = atm.tile([128, Nq], F16, tag="ms", name="ms")
                    nc.gpsimd.tensor_mul(m_s[:], xsq[:],
                                         qtab_t[0:128, Nq:2 * Nq])
                    prot0 = rot_pool.tile([128, Nq], F32, tag="prot",
                                          name="prot0")
                    nc.tensor.matmul(prot0[:], rt_t, m_s[:],
                                     start=True, stop=True)
                    nc.vector.tensor_add(
                        qpl[:, m * 1024:m * 1024 + 512], m_c[:], prot0[:])
                    prot1 = rot_pool.tile([128, Nq], F32, tag="prot",
                                          name="prot1")
                    nc.tensor.matmul(prot1[:], rt_t, m_c[:],
                                     start=True, stop=True)
                    nc.vector.tensor_sub(
                        qpl[:, m * 1024 + 512:(m + 1) * 1024],
                        m_s[:], prot1[:])

                def vproj_chunk(vb, scj, pvp):
                    vt_ = vt_ts[vb][:].rearrange("p (a n) -> p a n", n=512)
                    wv_ = wv_t[:].rearrange("p (a m) -> p a m", m=CH)
                    sc = vb * 4 + scj
                    pv = pvp.tile([128, CH], F32, tag="pv", name="pv")
                    for s in range(8):
                        ws = s % 4
                        nc.tensor.matmul(
                            pv[:],
                            vt_[:, 2 * s:2 * s + 2,
                                scj * 128:(scj + 1) * 128],
                            wv_[:, 2 * ws:2 * ws + 2, :],
                            start=(s == 0), stop=(s == 7), perf_mode=DR)
                    nc.vector.tensor_scalar_mul(
                        v65[:, sc * 520:(sc + 1) * 520
                            ].rearrange("p (n w) -> p n w",
                                        w=65)[:, :, 0:64],
                        pv[:].rearrange("p (n w) -> p n w", w=64),
                        1.0 / WS)

                def scores_h(sbi, h, scp):
                    m, r0 = h // 2, 64 * (h % 2)
                    stv = kpl[m][r0:r0 + 64, :].rearrange(
                        "p (two n) -> p two n", two=2)
                    mvv = qpl[r0:r0 + 64,
                              m * 1024:(m + 1) * 1024].rearrange(
                        "p (two n) -> p two n", two=2)
                    for jj in range(2):
                        psc = scp.tile([128, 1024], F32, tag="psc",
                                       name="psc")
                        for j2 in range(2):
                            sc = sbi * 4 + jj * 2 + j2
                            nc.tensor.matmul(
                                psc[:, j2 * 512:(j2 + 1) * 512],
                                stv[:, :, sc * 128:(sc + 1) * 128],
                                mvv, start=True, stop=True, perf_mode=DR)
                        pt = pt_view(sbi, h, jj)
                        idx = sbi * 16 + h * 2 + jj
                        if N_DVE_EXP > 0 and (
                                (idx < 48 and idx % 7 in (1, 4))
                                or idx in (49, 52, 55, 58)):
                            nc.vector._custom_dve(
                                EXP_POLY_C, out=pt, in0=psc[:],
                                s0=float(ESC / 16.0 * 0.5 ** 0.5),
                                s1=float(0.5 ** 0.5), imm2=0.5)
                        else:
                            nc.scalar.activation(pt, psc[:], EXP, scale=ESC)
                        pts[(sbi, h, jj)] = pt

                pts = {}
                with ExitStack() as phb:
                    scp = phb.enter_context(
                        tc.tile_pool(name="scp", bufs=2, space="PSUM"))
                    pps_stack = ExitStack()
                    pps = pps_stack.enter_context(
                        tc.tile_pool(name="pps", bufs=2, space="PSUM"))
                    # block 0: per-pair weave; block-1 k-proj rides along
                    with ExitStack() as ph0:
                        rp0 = ph0.enter_context(
                            tc.tile_pool(name="rp0", bufs=2, space="PSUM"))
                        for m in range(NPAIR):
                            kproj_m(0, m, pps)
                            qside_m(m, pps, rp0)
                            scores_h(0, 2 * m, scp)
                            scores_h(0, 2 * m + 1, scp)
                    s0.close()
                    nc.vector.memset(
                        v65[:].rearrange("p (s h w) -> p s h w", h=HL,
                                         w=65)[:, :, :, 64:65], 1.0)
                    wvp = vs.enter_context(tc.tile_pool(name="wvp", bufs=1))
                    vst = vs.enter_context(tc.tile_pool(name="vst", bufs=2))
                    wv_t = wvp.tile([128, 8 * CH], F8, tag="wv", name="wv")
                    _ld3(nc.sync, wv_t[:], wvT[:], CH)
                    for _sbi in range(SB):
                        _sl = slice(_sbi * 512, (_sbi + 1) * 512)
                        vt_t = vst.tile([128, 8192], F8, tag="vt", name="vt")
                        _ld3(nc.sync, vt_t[:, 0:4096], vhT[:, _sl], 512)
                        _ld3(nc.sync, vt_t[:, 4096:8192], vlT[:, _sl], 512)
                        vt_ts.append(vt_t)
                    # blocks 1-2: scores woven with next-block k-proj and
                    # the v projections (thunk queue, ~1-2 per head)
                    with ExitStack() as phk:
                        pvp = phk.enter_context(
                            tc.tile_pool(name="pvp", bufs=2, space="PSUM"))
                        for m in range(NPAIR):
                            kproj_m(1, m, pps)
                        weaves = {
                            1: ([lambda m=q // 2, hf=q % 2:
                                 kproj_half(2, m, pps, hf)
                                 for q in range(2 * NPAIR)]
                                + [lambda c=c: vproj_chunk(c // 4, c % 4,
                                                           pvp)
                                   for c in range(0, 8)]),
                            2: ([lambda c=c: vproj_chunk(c // 4, c % 4,
                                                         pvp)
                                 for c in range(8, 12)]
                                + [lambda q=q: kproj_half(3, q // 2, pps,
                                                          q % 2)
                                   for q in range(2 * NPAIR)]
                                + [lambda c=c: vproj_chunk(c // 4, c % 4,
                                                           pvp)
                                   for c in range(12, 16)]),
                        }
                        for sbi in (1, 2):
                            wq_ = weaves[sbi]
                            nper = (len(wq_) + HL - 1) // HL
                            for h in range(HL):
                                scores_h(sbi, h, scp)
                                for t in wq_[h * nper:(h + 1) * nper]:
                                    t()
                    vs.close()
                    stg.close()
                    pps_stack.close()
                    wpp = it.enter_context(tc.tile_pool(name="wpp", bufs=1))
                    wp_t = wpp.tile([128, NPAIR * 1024], F16, tag="wp",
                                    name="wp")
                    _ld3(nc.sync, wp_t[:], wpT[:], 1024)

                    # block-3 scores with PV/normalization/transposes woven
                    # in one head behind the exp wave
                    xtt = it.enter_context(
                        tc.tile_pool(name="xtt", bufs=1))
                    with ExitStack() as phx:
                        xtp = phx.enter_context(
                            tc.tile_pool(name="xtp", bufs=2, space="PSUM"))
                        xnT = [xtt.tile([128, Nq], F16, tag=f"xnT{p}",
                                        name=f"xnT{p}")
                               for p in range(NPAIR)]

                        def pv_h(h):
                            pxt = xtp.tile([128, 260], F32, tag="pxt",
                                           name="pxt",
                                           padded_shape=[128, 512])
                            for qc in range(4):
                                for sc in range(SC):
                                    sbi, jj, j2 = (sc // 4, (sc % 4) // 2,
                                                   sc % 2)
                                    nc.tensor.matmul(
                                        pxt[:, qc * 65:(qc + 1) * 65],
                                        pts[(sbi, h, jj)][
                                            :, j2 * 512 + qc * 128:
                                            j2 * 512 + (qc + 1) * 128],
                                        v65[:, sc * 520 + h * 65:
                                            sc * 520 + (h + 1) * 65],
                                        start=(sc == 0), stop=(sc == SC - 1))
                            nc.vector.reciprocal(
                                inv_t[:, h * 4:(h + 1) * 4].rearrange(
                                    "p (a b) -> p a b", b=1),
                                pxt[:].rearrange("p (q w) -> p q w",
                                                 w=65)[:, :, 64:65])
                            for qc in range(4):
                                nc.vector.tensor_scalar_mul(
                                    xn[:, (h * 4 + qc) * 64:
                                       (h * 4 + qc + 1) * 64],
                                    pxt[:, qc * 65:qc * 65 + 64],
                                    inv_t[:, h * 4 + qc:h * 4 + qc + 1])

                        def transp_p(p):
                            for qc in range(4):
                                ptf = xtp.tile([128, 256], F32, tag="pxt",
                                               name="ptr",
                                               padded_shape=[128, 512])
                                ptr = ptf[:].bitcast(F16)
                                for sub in range(2):
                                    hh = 2 * p + sub
                                    nc.tensor.transpose(
                                        ptr[sub * 64:(sub + 1) * 64, 0:128],
                                        xn[:, (hh * 4 + qc) * 64:
                                           (hh * 4 + qc + 1) * 64],
                                        id_t[:],
                                        tile_position=(0, sub * 64))
                                nc.vector.tensor_copy(
                                    xnT[p][:, qc * 128:(qc + 1) * 128],
                                    ptr[0:128, 0:128])

                        for h in range(HL):
                            scores_h(3, h, scp)
                            if h >= 1:
                                pv_h(h - 1)
                                if (h - 1) % 2 == 1:
                                    transp_p((h - 1) // 2)
                        pv_h(HL - 1)
                        transp_p(NPAIR - 1)

                if DEBUG:
                    nc.sync.dma_start(d_qpl[:], qpl[:])
                    nc.sync.dma_start(d_kpl[:], kpl[0][:])
                    nc.sync.dma_start(d_v65[:], v65[:])
                    nc.sync.dma_start(d_xn[:], xn[:])
                    nc.sync.dma_start(d_pt[:, 0:1024], pt_view(0, 0, 0))
                    nc.sync.dma_start(d_pt[:, 1024:2048], pt_view(3, 7, 1))
                # == out projection (all score/PV psum freed) ==
                with ExitStack() as tl:
                    pop = tl.enter_context(
                        tc.tile_pool(name="pop", bufs=3, space="PSUM"))
                    wp_ = wp_t[:].rearrange("p (a s) -> p a s", s=1024)
                    for j in range(8):
                        poq = pop.tile([128, 512], F32, tag="po", name="po")
                        for p in range(NPAIR):
                            nc.tensor.matmul(
                                poq[:],
                                wp_[:, p, j * 128:(j + 1) * 128],
                                xnT[p][:],
                                start=(p == 0), stop=(p == NPAIR - 1))
                        # osb space: reuse the dead kpl tiles (bitcast f32)
                        ob = kpl[j // 2][:].bitcast(F32)[
                            :, (j % 2) * 512:(j % 2 + 1) * 512]
                        if j % 2 == 0:
                            nc.scalar.copy(ob, poq[:])
                        else:
                            nc.vector.tensor_copy(ob, poq[:])
                        if j % 2 == 1:
                            obv = kpl[j // 2][:].bitcast(F32)[:, 0:1024]
                            nc.sync.dma_start(
                                outT[(j - 1) * 128:(j + 1) * 128,
                                     :].rearrange(
                                    "(a p) s -> p a s", p=128),
                                obv.rearrange("p (a s) -> p a s", s=512))

    nc.compile()
    return nc


def prep_inputs(query, key, value, qpos, kpos, Wq, Wk, Wv, Wp, bp):
    """Build per-core input maps (8 cores: core = 2*b + g)."""
    invf = (1.0 / ROPE_BASE ** (np.arange(0, D, 2, dtype=np.float32) / D)
            ).astype(np.float32)
    rows64 = invf[np.arange(64) % 32]          # [64]

    R64 = np.zeros((64, 64), dtype=np.float32)
    for r in range(32):
        R64[r, r + 32] = -1.0
        R64[r + 32, r] = 1.0
    rT128 = np.zeros((128, 128), dtype=np.float32)
    rT128[0:64, 0:64] = R64.T
    rT128[64:128, 64:128] = R64.T
    cst_np = np.concatenate(
        [rT128, np.eye(128, dtype=np.float32)], axis=1).astype(F16NP)

    def hilo(x):
        x = np.ascontiguousarray(x, dtype=np.float32)
        hi = x.astype(E4NP)
        lo = (x - hi.astype(np.float32)).astype(E4NP)
        return hi, lo

    qf = np.asarray(query, np.float32)
    kf = np.asarray(key, np.float32)
    vf = np.asarray(value, np.float32)
    q8 = {b: hilo(qf[b].T) for b in range(B)}
    k8 = {b: (np.ascontiguousarray(kf[b].T).astype(E4NP),) for b in range(B)}
    v8 = {b: hilo(vf[b].T) for b in range(B)}

    in_maps = []
    for core in range(8):
        b, g = core // 2, core % 2
        cols = slice(g * CH, (g + 1) * CH)
        qang = rows64[:, None] * np.asarray(qpos[b], np.float32)[None, :]
        kang = rows64[:, None] * np.asarray(kpos[b], np.float32)[None, :]
        qtab_np = np.concatenate(
            [np.tile(np.cos(qang), (2, 1)), np.tile(np.sin(qang), (2, 1))],
            axis=1).astype(F16NP)
        ktab_np = np.concatenate(
            [np.concatenate(
                [np.tile(np.cos(kang[:, s * 512:(s + 1) * 512]), (2, 1)),
                 np.tile(np.sin(kang[:, s * 512:(s + 1) * 512]), (2, 1))],
                axis=1)
             for s in range(SB)], axis=1).astype(F16NP)
        m = {
            "qhT": q8[b][0], "qlT": q8[b][1],
            "khT": k8[b][0],
            "vhT": v8[b][0], "vlT": v8[b][1],
            "wqT": np.ascontiguousarray(
                np.asarray(Wq, np.float32)[cols, :].T * WS).astype(E4NP),
            "wkT": np.ascontiguousarray(
                np.asarray(Wk, np.float32)[cols, :].T * WS).astype(E4NP),
            "wvT": np.ascontiguousarray(
                np.asarray(Wv, np.float32)[cols, :].T * WS).astype(E4NP),
            "wpT": np.ascontiguousarray(
                np.asarray(Wp, np.float32)[:, cols].T).astype(F16NP),
            "cst": cst_np,
            "qtab": qtab_np,
            "ktab": ktab_np,
        }
        in_maps.append(m)
    return in_maps


_NC_CACHE = {}


def _get_nc(iters=1):
    if iters not in _NC_CACHE:
        _NC_CACHE[iters] = build_nc(iters)
    return _NC_CACHE[iters]


def kernel(query, key, value, qpos, kpos, Wq, Wk, Wv, Wp, bp):
    from concourse.bass_utils import run_bass_kernel_spmd

    nc = _get_nc()
    in_maps = prep_inputs(query, key, value, qpos, kpos, Wq, Wk, Wv, Wp, bp)
    res = run_bass_kernel_spmd(nc, in_maps, list(range(8)))
    out = np.zeros((B, Nq, C), dtype=np.float32)
    for core in range(8):
        out[core // 2] += res.results[core]["outT"].T
    out += np.asarray(bp, np.float32)
    return out
